# revision 31
# baseline (speedup 1.0000x reference)
"""Distributed multi-head attention (RoPE + SDPA + out-proj) for one TRN2 chip.

Sharding (v3): 8 cores = 4 batches x 2 head-halves (tensor parallel over
heads).  Each core computes Q/K/V projections for its 8 heads over the FULL
sequence (no duplicated K/V work, unlike the q-half split), attention for
those heads, and a partial output projection against its row-block of wo.
The host sums the two partial outputs per batch (the "all-reduce" of the
TP sharding, done for free in the unshard step).

Per-core structure:
  - 4 head-pair tiles (ct); heads 2ct/2ct+1 packed at partitions 0-63/64-127
  - padded per-head K tiles (krA/krB) so score matmuls run full C=128
    stationaries (LDWEIGHTS fast path) as in v2
  - attention unit u = (ct, qh, half): 16 kt score matmuls into fp32 PSUM,
    one Exp per kt over [128,1024], PV accumulation with a ones-column in V
    producing the softmax denominator at partition 64
  - software pipeline: PV(u) is emitted one unit behind scores(u+1), with
    K/Q projections of later cts and the output projection as PE filler, so
    the PE stream never waits on ACT exp results
  - PSUM→SBUF evictions and rope run on DVE; exp owns ACT; reciprocal
    broadcast on Pool; inputs split across engine DMA queues
"""

from contextlib import ExitStack, nullcontext

import ml_dtypes
import numpy as np

import concourse.bass as bass
import concourse.tile as tile
from concourse import bacc, bass_utils, mybir

B, S, D, H = 4, 2048, 1024, 16
DH = D // H
NCORES = 8
HL = H // 2          # 8 local heads per core
DL = HL * DH         # 512 local model dim
BF = mybir.dt.bfloat16
F32 = mybir.dt.float32
BF_NP = ml_dtypes.bfloat16

KT_D = D // 128      # 8  d-tiles (contraction for QKV projections)
KT_S = S // 128      # 16 seq tiles
KT_DL = DL // 128    # 4  local-d tiles (contraction for out projection)
NCT = KT_DL          # 4  head-pair tiles
NS = S // 512        # 4  proj chunks over full seq


def _build(reps=1):
    nc = bacc.Bacc("TRN2", target_bir_lowering=False, debug=False,
                   num_devices=NCORES)

    xT = nc.dram_tensor("xT", [D, S], BF, kind="ExternalInput").ap()
    wqT = nc.dram_tensor("wqT", [D, DL], BF, kind="ExternalInput").ap()
    wkT = nc.dram_tensor("wkT", [D, DL], BF, kind="ExternalInput").ap()
    wvT = nc.dram_tensor("wvT", [D, DL], BF, kind="ExternalInput").ap()
    woT = nc.dram_tensor("woT", [DL, D], BF, kind="ExternalInput").ap()
    cosk = nc.dram_tensor("cosk", [128, S], BF, kind="ExternalInput").ap()
    sinrk = nc.dram_tensor("sinrk", [128, S], BF, kind="ExternalInput").ap()
    out = nc.dram_tensor("out", [S, D], BF, kind="ExternalOutput").ap()

    with tile.TileContext(nc) as tc, \
         (tc.For_i(0, reps) if reps > 1 else nullcontext()), \
         ExitStack() as ctx:
        qr_pool = ctx.enter_context(tc.tile_pool(name="qr", bufs=2))
        kr_pool = ctx.enter_context(tc.tile_pool(name="kr", bufs=2))
        v_pool = ctx.enter_context(tc.tile_pool(name="v", bufs=KT_S))
        ctxT_pool = ctx.enter_context(tc.tile_pool(name="ctxT", bufs=NCT))
        x_pool = ctx.enter_context(tc.tile_pool(name="x", bufs=1))
        w_pool = ctx.enter_context(tc.tile_pool(name="w", bufs=1))
        wkq_pool = ctx.enter_context(tc.tile_pool(name="wkq", bufs=2 * NCT))
        raw_pool = ctx.enter_context(tc.tile_pool(name="raw", bufs=2))
        rot_pool = ctx.enter_context(tc.tile_pool(name="rot", bufs=1))
        tab_pool = ctx.enter_context(tc.tile_pool(name="tab", bufs=1))
        exp_pool = ctx.enter_context(tc.tile_pool(name="exp", bufs=24))
        rc_pool = ctx.enter_context(tc.tile_pool(name="rc", bufs=2))
        osb_pool = ctx.enter_context(tc.tile_pool(name="osb", bufs=2))
        psA = ctx.enter_context(tc.tile_pool(name="psA", bufs=2, space="PSUM"))
        psS = ctx.enter_context(tc.tile_pool(name="psS", bufs=2, space="PSUM"))
        psC = ctx.enter_context(tc.tile_pool(name="psC", bufs=2, space="PSUM"))

        # ---- input DMAs: one queue, priority order (the DMA engines
        # drain in arrival order, so critical tiles go first): wk0/wq0,
        # then x k-tile by k-tile so the ct0 projections can chase the
        # transfers, tables for rope, then the rest ----
        x_all = x_pool.tile([128, KT_D * S], BF, tag="x", name="x_all")
        x_t = [x_all[:, i * S:(i + 1) * S] for i in range(KT_D)]
        x_src = xT.rearrange("(i p) c -> p i c", p=128)
        x_dst = x_all[:].rearrange("p (i c) -> p i c", c=S)

        wk_ct = [None] * NCT
        wq_ct = [None] * NCT
        cosk_t = tab_pool.tile([128, S], BF, tag="ck")
        sink_t = tab_pool.tile([128, S], BF, tag="sk")

        def load_w_ct(w_dram, ct, lst):
            t = wkq_pool.tile([128, D], BF, tag="wkq", name="wkq")
            src = w_dram[:, ct * 128:(ct + 1) * 128].rearrange(
                "(k p) c -> p k c", p=128)
            nc.sync.dma_start(t[:].rearrange("p (k c) -> p k c", c=128),
                              src)
            lst[ct] = t

        load_w_ct(wkT, 0, wk_ct)
        load_w_ct(wqT, 0, wq_ct)
        for k in range(KT_D):
            nc.sync.dma_start(x_dst[:, k:k + 1], x_src[:, k:k + 1])
        nc.sync.dma_start(cosk_t[:], cosk[:])
        nc.sync.dma_start(sink_t[:], sinrk[:])
        wv_all = w_pool.tile([128, KT_D * DL], BF, tag="wv", name="wv_all")
        wv_t = [wv_all[:, i * DL:(i + 1) * DL] for i in range(KT_D)]
        nc.sync.dma_start(
            wv_all[:].rearrange("p (i c) -> p i c", c=DL),
            wvT.rearrange("(i p) c -> p i c", p=128))
        for ct in range(1, NCT):
            load_w_ct(wkT, ct, wk_ct)
            load_w_ct(wqT, ct, wq_ct)
        # wo last: under the reps loop its transfer gates on the previous
        # iteration's O-proj reads, and nothing may queue behind it
        wo_all = w_pool.tile([128, KT_DL * D], BF, tag="wo", name="wo_all")
        wo_t = [wo_all[:, i * D:(i + 1) * D] for i in range(KT_DL)]
        nc.sync.dma_start(
            wo_all[:].rearrange("p (i c) -> p i c", c=D),
            woT.rearrange("(i p) c -> p i c", p=128))

        v_t = [v_pool.tile([128, HL * (DH + 1)], BF, tag="v", name="v")
               for _ in range(KT_S)]
        ctxT_t = [ctxT_pool.tile([128, S], BF, tag="ctxT", name="ctxT")
                  for _ in range(NCT)]
        for i in range(KT_S):
            # only the per-head ones-columns need initializing; data columns
            # are fully overwritten by the V-projection eviction copies
            vcol = v_t[i][:].rearrange("p (h c) -> p h c", c=DH + 1)
            nc.gpsimd.memset(vcol[:, :, DH:DH + 1], 1.0)

        def project_rope(w_ct, o, o_hi=None, chunked=False):
            """o = rope(w_ct.T @ x) for one 128-row o-tile over full S.

            With o_hi given, the two 64-row halves are written to o[0:64]
            and o_hi[64:128] (split per-head tiles whose other halves stay
            zero, so score matmuls can run full C=128 contraction).
            chunked=True ropes each 512-col chunk right after its PSUM
            eviction (shortens the DVE critical path for ct0)."""
            raw = raw_pool.tile([128, S], BF, tag="raw")
            rot = rot_pool.tile([128, S], BF, tag="rot")
            halves = ((o, 0), (o if o_hi is None else o_hi, 64))

            def rope_cols(cs):
                for b0 in (0, 64):
                    nc.vector.tensor_copy(rot[b0:b0 + 32, cs],
                                          raw[b0 + 32:b0 + 64, cs])
                    nc.vector.tensor_copy(rot[b0 + 32:b0 + 64, cs],
                                          raw[b0:b0 + 32, cs])
                nc.vector.tensor_mul(rot[:, cs], rot[:, cs], sink_t[:, cs])
                for dst, b0 in halves:
                    sl = slice(b0, b0 + 64)
                    nc.vector.tensor_mul(dst[sl, cs], raw[sl, cs],
                                         cosk_t[sl, cs])
                    nc.vector.tensor_add(dst[sl, cs], dst[sl, cs],
                                         rot[sl, cs])

            for n in range(NS):
                cs = slice(n * 512, (n + 1) * 512)
                ps = psA.tile([128, 512], F32, tag="psA")
                for k in range(KT_D):
                    nc.tensor.matmul(
                        ps[:], w_ct[:, k * 128:(k + 1) * 128],
                        x_t[k][:, n * 512:(n + 1) * 512],
                        start=(k == 0), stop=(k == KT_D - 1))
                nc.vector.tensor_copy(raw[:, cs], ps[:])
                if chunked:
                    rope_cols(cs)
            if not chunked:
                rope_cols(slice(0, S))

        def v_proj(m):
            """v[m] = x.T @ wvT for seq tile m, strided into 65-col blocks."""
            ps = psA.tile([128, 512], F32, tag="psA")
            for k in range(KT_D):
                nc.tensor.matmul(
                    ps[:], x_t[k][:, m * 128:(m + 1) * 128], wv_t[k][:],
                    start=(k == 0), stop=(k == KT_D - 1))
            dst = v_t[m][:].rearrange("p (h c) -> p h c", c=DH + 1)
            src = ps[:].rearrange("p (h c) -> p h c", c=DH)
            nc.vector.tensor_copy(dst[:, :, 0:DH], src[:])

        def kq_proj(ct):
            """K and Q projections + rope for head-pair ct."""
            krA = kr_pool.tile([128, S], BF, tag="krA", name="krA")
            krB = kr_pool.tile([128, S], BF, tag="krB", name="krB")
            if ct < 2:
                # zero the pad halves once per pool slot; later cts reuse
                # the slots and nothing rewrites the pad rows
                nc.gpsimd.memset(krA[64:128, :], 0.0)
                nc.gpsimd.memset(krB[0:64, :], 0.0)
            qr = qr_pool.tile([128, S], BF, tag="qr", name="qr")
            project_rope(wk_ct[ct], krA, o_hi=krB, chunked=(ct == 0))
            project_rope(wq_ct[ct], qr, chunked=(ct == 0))
            return krA, krB, qr

        def o_proj(m, evict_act=True, deep=False):
            """out[m*128:(m+1)*128, :] = ctxT.T @ woT for seq tile m.

            PSUM groups rotate across the psA/psS pools (exp-free by the
            time O-proj runs; psC joins once the last normalize is done)
            for a 4-6 deep pipeline.  evict_act=False routes the PSUM
            eviction to DVE for the stretch where ACT is still busy with
            the last exps."""
            rotation = ((psA, "psA"), (psS, "psS"))
            if deep:
                rotation = ((psA, "psA"), (psS, "psS"), (psC, "psC"))
            for n in range(2):
                pool, tg = rotation[(2 * m + n) % len(rotation)]
                ps = pool.tile([128, 512], F32, tag=tg, name="psO")
                for k in range(KT_DL):
                    nc.tensor.matmul(
                        ps[:], ctxT_t[k][:, m * 128:(m + 1) * 128],
                        wo_t[k][:, n * 512:(n + 1) * 512],
                        start=(k == 0), stop=(k == KT_DL - 1))
                ot = osb_pool.tile([128, 512], BF, tag="osb")
                if evict_act:
                    nc.scalar.activation(ot[:], ps[:],
                                         mybir.ActivationFunctionType.Copy)
                else:
                    nc.vector.tensor_copy(ot[:], ps[:])
                # out DMAs stay off the sync queue: under the reps loop the
                # sync queue must reach the next iteration's input DMAs
                # early so x/weights prefetch behind the current tail
                eng = nc.scalar if (m + n) % 2 == 0 else nc.gpsimd
                eng.dma_start(
                    out[m * 128:(m + 1) * 128, n * 512:(n + 1) * 512], ot[:])

        # ---- attention unit: scores+exp for (ct, qh, half) with the
        # PREVIOUS unit's PV matmuls interleaved at kt granularity.  The
        # scores stream alone is throttled by ACT (psS bufs=2, exp at
        # ~1040ns/kt vs 850ns/kt of score matmuls); the interleaved PV
        # work keeps PE busy while ACT drains the score PSUMs. ----
        def scores_pv(u, prev):
            ct, qh, half, krA, krB, qr = u
            q0 = qh * 1024
            krh = krA if half == 0 else krB
            if prev is not None:
                (pct, pqh, phalf, _, _, _), pexpt = prev
                ph = 2 * pct + phalf
                cpss = [psC.tile([65, 512], F32, tag="psC", name="cps")
                        for _ in range(2)]
            expt = []
            for kt in range(KT_S):
                if prev is not None:
                    for qb in range(2):
                        nc.tensor.matmul(
                            cpss[qb][:],
                            v_t[kt][:, ph * (DH + 1):(ph + 1) * (DH + 1)],
                            pexpt[kt][:, qb * 512:(qb + 1) * 512],
                            start=(kt == 0), stop=(kt == KT_S - 1))
                pss = psS.tile([128, 1024], F32, tag="psS", name="pss")
                st = krh[:, kt * 128:(kt + 1) * 128]
                for qb in range(2):
                    nc.tensor.matmul(
                        pss[:, qb * 512:(qb + 1) * 512], st,
                        qr[:, q0 + qb * 512:q0 + (qb + 1) * 512],
                        start=True, stop=True)
                et = exp_pool.tile([128, 1024], BF, tag="exp", name="exp")
                nc.scalar.activation(et[:], pss[:],
                                     mybir.ActivationFunctionType.Exp,
                                     scale=0.125)
                expt.append(et)
            if prev is not None:
                norm(prev[0], cpss)
            return expt

        def pv_last(u, expt):
            ct, qh, half, krA, krB, qr = u
            h = 2 * ct + half
            cpss = [psC.tile([65, 512], F32, tag="psC", name="cps")
                    for _ in range(2)]
            for kt in range(KT_S):
                for qb in range(2):
                    nc.tensor.matmul(
                        cpss[qb][:],
                        v_t[kt][:, h * (DH + 1):(h + 1) * (DH + 1)],
                        expt[kt][:, qb * 512:(qb + 1) * 512],
                        start=(kt == 0), stop=(kt == KT_S - 1))
            norm(u, cpss)

        def norm(u, cpss):
            ct, qh, half, krA, krB, qr = u
            b0 = half * 64
            q0 = qh * 1024
            for qb in range(2):
                cps = cpss[qb]
                # reciprocal lands at partition 0: the gpsimd broadcast
                # hardware reads the source on Q7 core 0 and pushes right,
                # so the source row must live in partition 0
                rc = rc_pool.tile([1, 512], BF, tag="rc")
                with nc.allow_low_precision(reason="bf16 softmax denom"):
                    nc.vector.reciprocal(rc[0:1, :], cps[64:65, :])
                bcs = rc_pool.tile([64, 512], BF, tag="bcs", name="bcs")
                nc.gpsimd.partition_broadcast(bcs[:], rc[0:1, :],
                                              channels=64)
                nc.vector.tensor_mul(
                    ctxT_t[ct][b0:b0 + 64,
                               q0 + qb * 512:q0 + (qb + 1) * 512],
                    cps[0:64, :], bcs[:])

        # ---- emission schedule ----
        # prologue: projections for ct0 + all of V (PE fills while ACT
        # warms up); then the 16-unit pipeline with PV one unit behind and
        # K/Q projections of later cts + out projection as PE filler.
        kq = [None] * NCT
        kq[0] = kq_proj(0)
        # V tiles 0-5 fill the PE while rope(ct0) runs on DVE; the rest of
        # V is spread over steps 0-1 so ACT's exp stream starts ~25us
        # earlier.  All of V must be emitted before PV(u0) (step 1 post).
        for m in range(0, 6):
            v_proj(m)

        units = [(ct, qh, half) for ct in range(NCT) for qh in range(2)
                 for half in range(2)]
        # filler emitted after scores_pv(u_i), keyed by pipeline step i.
        # All of V must precede step 1 (PV(u0) consumes every v tile).
        filler = {
            0: lambda: [v_proj(m) for m in range(6, KT_S)],
            1: lambda: kq.__setitem__(1, kq_proj(1)),
            4: lambda: kq.__setitem__(2, kq_proj(2)),
            8: lambda: kq.__setitem__(3, kq_proj(3)),
        }
        prev = None
        for i, (ct, qh, half) in enumerate(units):
            krA, krB, qr = kq[ct]
            u = (ct, qh, half, krA, krB, qr)
            expt = scores_pv(u, prev)
            if f := filler.get(i):
                f()
            prev = (u, expt)
        # O(m0-7) needs unit 13's normalize (emitted inside step 14) and
        # covers the exp latency of unit 15 before its PV runs.
        for m in range(0, 8):
            o_proj(m, evict_act=False)
        pv_last(*prev)
        for m in range(8, KT_S):
            o_proj(m, deep=True)

    nc.compile()
    return nc


_NC = None
LAST_RESULT = None
LAST_IN_MAPS = None


def _get_nc():
    global _NC
    if _NC is None:
        _NC = _build()
    return _NC


def unshard(results):
    """Sum the per-core bf16 partial outputs into the full [B, S, D]."""
    out_full = np.empty((B, S, D), np.float32)
    for b in range(B):
        out_full[b] = (results[2 * b]["out"].astype(np.float32) +
                       results[2 * b + 1]["out"].astype(np.float32))
    return out_full


def kernel(x, cos, sin, wq, wk, wv, wo):
    global LAST_RESULT, LAST_IN_MAPS
    x = np.asarray(x)
    cos = np.asarray(cos)
    sin = np.asarray(sin)

    def bf(a):
        return np.ascontiguousarray(a, dtype=np.float32).astype(BF_NP)

    cosT = cos[0, :, 0, :].T.astype(np.float32)   # [DH, S]
    sinT = sin[0, :, 0, :].T.astype(np.float32)
    sinr = np.concatenate([-sinT[:DH // 2], sinT[DH // 2:]], axis=0)
    cos2 = bf(np.concatenate([cosT, cosT], axis=0))   # [128, S]
    sinr2 = bf(np.concatenate([sinr, sinr], axis=0))

    wqT, wkT, wvT, woT = (w.T.astype(np.float32) for w in (wq, wk, wv, wo))
    in_maps = []
    for c in range(NCORES):
        b, hh = c // 2, c % 2
        d0 = hh * DL
        in_maps.append({
            "xT": bf(x[b].T),
            "wqT": bf(wqT[:, d0:d0 + DL]),
            "wkT": bf(wkT[:, d0:d0 + DL]),
            "wvT": bf(wvT[:, d0:d0 + DL]),
            "woT": bf(woT[d0:d0 + DL, :]),
            "cosk": cos2, "sinrk": sinr2,
        })

    LAST_IN_MAPS = in_maps
    nc = _get_nc()
    res = bass_utils.run_bass_kernel_spmd(nc, in_maps,
                                          core_ids=list(range(NCORES)))
    LAST_RESULT = res
    return unshard(res.results)


# revision 32
# speedup vs baseline: 1.5671x; 1.5671x over previous
"""Distributed multi-head attention (RoPE + SDPA + out-proj) for one TRN2 chip.

Sharding (v3): 8 cores = 4 batches x 2 head-halves (tensor parallel over
heads).  Each core computes Q/K/V projections for its 8 heads over the FULL
sequence (no duplicated K/V work, unlike the q-half split), attention for
those heads, and a partial output projection against its row-block of wo.
The host sums the two partial outputs per batch (the "all-reduce" of the
TP sharding, done for free in the unshard step).

Per-core structure:
  - 4 head-pair tiles (ct); heads 2ct/2ct+1 packed at partitions 0-63/64-127
  - padded per-head K tiles (krA/krB) so score matmuls run full C=128
    stationaries (LDWEIGHTS fast path) as in v2
  - attention unit u = (ct, qh, half): 16 kt score matmuls into fp32 PSUM,
    one Exp per kt over [128,1024], PV accumulation with a ones-column in V
    producing the softmax denominator at partition 64
  - software pipeline: PV(u) is emitted one unit behind scores(u+1), with
    K/Q projections of later cts and the output projection as PE filler, so
    the PE stream never waits on ACT exp results
  - PSUM→SBUF evictions and rope run on DVE; exp owns ACT; reciprocal
    broadcast on Pool; inputs split across engine DMA queues
"""

from contextlib import ExitStack, nullcontext

import ml_dtypes
import numpy as np

import concourse.bass as bass
import concourse.tile as tile
from concourse import bacc, bass_utils, mybir

B, S, D, H = 4, 2048, 1024, 16
DH = D // H
NCORES = 8
HL = H // 2          # 8 local heads per core
DL = HL * DH         # 512 local model dim
BF = mybir.dt.bfloat16
F32 = mybir.dt.float32
BF_NP = ml_dtypes.bfloat16

KT_D = D // 128      # 8  d-tiles (contraction for QKV projections)
KT_S = S // 128      # 16 seq tiles
KT_DL = DL // 128    # 4  local-d tiles (contraction for out projection)
NCT = KT_DL          # 4  head-pair tiles
NS = S // 512        # 4  proj chunks over full seq


def _build(reps=1):
    nc = bacc.Bacc("TRN2", target_bir_lowering=False, debug=False,
                   num_devices=NCORES)

    xT = nc.dram_tensor("xT", [D, S], BF, kind="ExternalInput").ap()
    wqT = nc.dram_tensor("wqT", [D, DL], BF, kind="ExternalInput").ap()
    wkT = nc.dram_tensor("wkT", [D, DL], BF, kind="ExternalInput").ap()
    wvT = nc.dram_tensor("wvT", [D, DL], BF, kind="ExternalInput").ap()
    woT = nc.dram_tensor("woT", [DL, D], BF, kind="ExternalInput").ap()
    cosk = nc.dram_tensor("cosk", [128, S], BF, kind="ExternalInput").ap()
    sinrk = nc.dram_tensor("sinrk", [128, S], BF, kind="ExternalInput").ap()
    out = nc.dram_tensor("out", [S, D], BF, kind="ExternalOutput").ap()

    with tile.TileContext(nc) as tc, \
         (tc.For_i(0, reps) if reps > 1 else nullcontext()), \
         ExitStack() as ctx:
        qr_pool = ctx.enter_context(tc.tile_pool(name="qr", bufs=2))
        kr_pool = ctx.enter_context(tc.tile_pool(name="kr", bufs=2))
        v_pool = ctx.enter_context(tc.tile_pool(name="v", bufs=KT_S))
        ctxT_pool = ctx.enter_context(tc.tile_pool(name="ctxT", bufs=NCT))
        x_pool = ctx.enter_context(tc.tile_pool(name="x", bufs=1))
        w_pool = ctx.enter_context(tc.tile_pool(name="w", bufs=1))
        wkq_pool = ctx.enter_context(tc.tile_pool(name="wkq", bufs=2 * NCT))
        raw_pool = ctx.enter_context(tc.tile_pool(name="raw", bufs=2))
        rot_pool = ctx.enter_context(tc.tile_pool(name="rot", bufs=1))
        tab_pool = ctx.enter_context(tc.tile_pool(name="tab", bufs=1))
        exp_pool = ctx.enter_context(tc.tile_pool(name="exp", bufs=24))
        rc_pool = ctx.enter_context(tc.tile_pool(name="rc", bufs=2))
        osb_pool = ctx.enter_context(tc.tile_pool(name="osb", bufs=2))
        psA = ctx.enter_context(tc.tile_pool(name="psA", bufs=2, space="PSUM"))
        psS = ctx.enter_context(tc.tile_pool(name="psS", bufs=2, space="PSUM"))
        psC = ctx.enter_context(tc.tile_pool(name="psC", bufs=2, space="PSUM"))

        # ---- input DMAs: one queue, priority order (the DMA engines
        # drain in arrival order, so critical tiles go first): wk0/wq0,
        # then x k-tile by k-tile so the ct0 projections can chase the
        # transfers, tables for rope, then the rest ----
        x_all = x_pool.tile([128, KT_D * S], BF, tag="x", name="x_all")
        x_t = [x_all[:, i * S:(i + 1) * S] for i in range(KT_D)]
        x_src = xT.rearrange("(i p) c -> p i c", p=128)
        x_dst = x_all[:].rearrange("p (i c) -> p i c", c=S)

        wk_ct = [None] * NCT
        wq_ct = [None] * NCT
        cosk_t = tab_pool.tile([128, S], BF, tag="ck")
        sink_t = tab_pool.tile([128, S], BF, tag="sk")

        def load_w_ct(w_dram, ct, lst):
            t = wkq_pool.tile([128, D], BF, tag="wkq", name="wkq")
            src = w_dram[:, ct * 128:(ct + 1) * 128].rearrange(
                "(k p) c -> p k c", p=128)
            nc.sync.dma_start(t[:].rearrange("p (k c) -> p k c", c=128),
                              src)
            lst[ct] = t

        load_w_ct(wkT, 0, wk_ct)
        load_w_ct(wqT, 0, wq_ct)
        for k in range(KT_D):
            nc.sync.dma_start(x_dst[:, k:k + 1], x_src[:, k:k + 1])
        nc.sync.dma_start(cosk_t[:], cosk[:])
        nc.sync.dma_start(sink_t[:], sinrk[:])
        wv_all = w_pool.tile([128, KT_D * DL], BF, tag="wv", name="wv_all")
        wv_t = [wv_all[:, i * DL:(i + 1) * DL] for i in range(KT_D)]
        nc.sync.dma_start(
            wv_all[:].rearrange("p (i c) -> p i c", c=DL),
            wvT.rearrange("(i p) c -> p i c", p=128))
        for ct in range(1, NCT):
            load_w_ct(wkT, ct, wk_ct)
            load_w_ct(wqT, ct, wq_ct)
        # wo last: under the reps loop its transfer gates on the previous
        # iteration's O-proj reads, and nothing may queue behind it
        wo_all = w_pool.tile([128, KT_DL * D], BF, tag="wo", name="wo_all")
        wo_t = [wo_all[:, i * D:(i + 1) * D] for i in range(KT_DL)]
        nc.sync.dma_start(
            wo_all[:].rearrange("p (i c) -> p i c", c=D),
            woT.rearrange("(i p) c -> p i c", p=128))

        v_t = [v_pool.tile([128, HL * (DH + 1)], BF, tag="v", name="v")
               for _ in range(KT_S)]
        ctxT_t = [ctxT_pool.tile([128, S], BF, tag="ctxT", name="ctxT")
                  for _ in range(NCT)]
        for i in range(KT_S):
            # only the per-head ones-columns need initializing; data columns
            # are fully overwritten by the V-projection eviction copies
            vcol = v_t[i][:].rearrange("p (h c) -> p h c", c=DH + 1)
            nc.gpsimd.memset(vcol[:, :, DH:DH + 1], 1.0)

        def project_rope(w_ct, o, o_hi=None, chunked=False):
            """o = rope(w_ct.T @ x) for one 128-row o-tile over full S.

            With o_hi given, the two 64-row halves are written to o[0:64]
            and o_hi[64:128] (split per-head tiles whose other halves stay
            zero, so score matmuls can run full C=128 contraction).
            chunked=True ropes each 512-col chunk right after its PSUM
            eviction (shortens the DVE critical path for ct0)."""
            raw = raw_pool.tile([128, S], BF, tag="raw")
            rot = rot_pool.tile([128, S], BF, tag="rot")
            halves = ((o, 0), (o if o_hi is None else o_hi, 64))

            def rope_cols(cs):
                for b0 in (0, 64):
                    nc.vector.tensor_copy(rot[b0:b0 + 32, cs],
                                          raw[b0 + 32:b0 + 64, cs])
                    nc.vector.tensor_copy(rot[b0 + 32:b0 + 64, cs],
                                          raw[b0:b0 + 32, cs])
                nc.vector.tensor_mul(rot[:, cs], rot[:, cs], sink_t[:, cs])
                for dst, b0 in halves:
                    sl = slice(b0, b0 + 64)
                    nc.vector.tensor_mul(dst[sl, cs], raw[sl, cs],
                                         cosk_t[sl, cs])
                    nc.vector.tensor_add(dst[sl, cs], dst[sl, cs],
                                         rot[sl, cs])

            for n in range(NS):
                cs = slice(n * 512, (n + 1) * 512)
                ps = psA.tile([128, 512], F32, tag="psA")
                for k in range(KT_D):
                    nc.tensor.matmul(
                        ps[:], w_ct[:, k * 128:(k + 1) * 128],
                        x_t[k][:, n * 512:(n + 1) * 512],
                        start=(k == 0), stop=(k == KT_D - 1))
                nc.vector.tensor_copy(raw[:, cs], ps[:])
                if chunked:
                    rope_cols(cs)
            if not chunked:
                rope_cols(slice(0, S))

        def v_proj(m):
            """v[m] = x.T @ wvT for seq tile m, strided into 65-col blocks."""
            ps = psA.tile([128, 512], F32, tag="psA")
            for k in range(KT_D):
                nc.tensor.matmul(
                    ps[:], x_t[k][:, m * 128:(m + 1) * 128], wv_t[k][:],
                    start=(k == 0), stop=(k == KT_D - 1))
            dst = v_t[m][:].rearrange("p (h c) -> p h c", c=DH + 1)
            src = ps[:].rearrange("p (h c) -> p h c", c=DH)
            nc.vector.tensor_copy(dst[:, :, 0:DH], src[:])

        def kq_proj(ct):
            """K and Q projections + rope for head-pair ct."""
            krA = kr_pool.tile([128, S], BF, tag="krA", name="krA")
            krB = kr_pool.tile([128, S], BF, tag="krB", name="krB")
            if ct < 2:
                # zero the pad halves once per pool slot; later cts reuse
                # the slots and nothing rewrites the pad rows
                nc.gpsimd.memset(krA[64:128, :], 0.0)
                nc.gpsimd.memset(krB[0:64, :], 0.0)
            qr = qr_pool.tile([128, S], BF, tag="qr", name="qr")
            project_rope(wk_ct[ct], krA, o_hi=krB, chunked=(ct == 0))
            project_rope(wq_ct[ct], qr, chunked=(ct == 0))
            return krA, krB, qr

        def o_proj(m, evict_act=True, deep=False):
            """out[m*128:(m+1)*128, :] = ctxT.T @ woT for seq tile m.

            PSUM groups rotate across the psA/psS pools (exp-free by the
            time O-proj runs; psC joins once the last normalize is done)
            for a 4-6 deep pipeline.  evict_act=False routes the PSUM
            eviction to DVE for the stretch where ACT is still busy with
            the last exps."""
            rotation = ((psA, "psA"), (psS, "psS"))
            if deep:
                rotation = ((psA, "psA"), (psS, "psS"), (psC, "psC"))
            for n in range(2):
                pool, tg = rotation[(2 * m + n) % len(rotation)]
                ps = pool.tile([128, 512], F32, tag=tg, name="psO")
                for k in range(KT_DL):
                    nc.tensor.matmul(
                        ps[:], ctxT_t[k][:, m * 128:(m + 1) * 128],
                        wo_t[k][:, n * 512:(n + 1) * 512],
                        start=(k == 0), stop=(k == KT_DL - 1))
                ot = osb_pool.tile([128, 512], BF, tag="osb")
                if evict_act:
                    nc.scalar.activation(ot[:], ps[:],
                                         mybir.ActivationFunctionType.Copy)
                else:
                    nc.vector.tensor_copy(ot[:], ps[:])
                eng = nc.sync if (m + n) % 2 == 0 else nc.gpsimd
                eng.dma_start(
                    out[m * 128:(m + 1) * 128, n * 512:(n + 1) * 512], ot[:])

        # ---- attention unit: scores+exp for (ct, qh, half) with the
        # PREVIOUS unit's PV matmuls interleaved at kt granularity.  The
        # scores stream alone is throttled by ACT (psS bufs=2, exp at
        # ~1040ns/kt vs 850ns/kt of score matmuls); the interleaved PV
        # work keeps PE busy while ACT drains the score PSUMs. ----
        def scores_pv(u, prev):
            ct, qh, half, krA, krB, qr = u
            q0 = qh * 1024
            krh = krA if half == 0 else krB
            if prev is not None:
                (pct, pqh, phalf, _, _, _), pexpt = prev
                ph = 2 * pct + phalf
                cpss = [psC.tile([65, 512], F32, tag="psC", name="cps")
                        for _ in range(2)]
            expt = []
            for kt in range(KT_S):
                if prev is not None:
                    for qb in range(2):
                        nc.tensor.matmul(
                            cpss[qb][:],
                            v_t[kt][:, ph * (DH + 1):(ph + 1) * (DH + 1)],
                            pexpt[kt][:, qb * 512:(qb + 1) * 512],
                            start=(kt == 0), stop=(kt == KT_S - 1))
                pss = psS.tile([128, 1024], F32, tag="psS", name="pss")
                st = krh[:, kt * 128:(kt + 1) * 128]
                for qb in range(2):
                    nc.tensor.matmul(
                        pss[:, qb * 512:(qb + 1) * 512], st,
                        qr[:, q0 + qb * 512:q0 + (qb + 1) * 512],
                        start=True, stop=True)
                et = exp_pool.tile([128, 1024], BF, tag="exp", name="exp")
                nc.scalar.activation(et[:], pss[:],
                                     mybir.ActivationFunctionType.Exp,
                                     scale=0.125)
                expt.append(et)
            if prev is not None:
                norm(prev[0], cpss)
            return expt

        def pv_last(u, expt):
            ct, qh, half, krA, krB, qr = u
            h = 2 * ct + half
            cpss = [psC.tile([65, 512], F32, tag="psC", name="cps")
                    for _ in range(2)]
            for kt in range(KT_S):
                for qb in range(2):
                    nc.tensor.matmul(
                        cpss[qb][:],
                        v_t[kt][:, h * (DH + 1):(h + 1) * (DH + 1)],
                        expt[kt][:, qb * 512:(qb + 1) * 512],
                        start=(kt == 0), stop=(kt == KT_S - 1))
            norm(u, cpss)

        def norm(u, cpss):
            ct, qh, half, krA, krB, qr = u
            b0 = half * 64
            q0 = qh * 1024
            for qb in range(2):
                cps = cpss[qb]
                # reciprocal lands at partition 0: the gpsimd broadcast
                # hardware reads the source on Q7 core 0 and pushes right,
                # so the source row must live in partition 0
                rc = rc_pool.tile([1, 512], BF, tag="rc")
                with nc.allow_low_precision(reason="bf16 softmax denom"):
                    nc.vector.reciprocal(rc[0:1, :], cps[64:65, :])
                bcs = rc_pool.tile([64, 512], BF, tag="bcs", name="bcs")
                nc.gpsimd.partition_broadcast(bcs[:], rc[0:1, :],
                                              channels=64)
                nc.vector.tensor_mul(
                    ctxT_t[ct][b0:b0 + 64,
                               q0 + qb * 512:q0 + (qb + 1) * 512],
                    cps[0:64, :], bcs[:])

        # ---- emission schedule ----
        # prologue: projections for ct0 + all of V (PE fills while ACT
        # warms up); then the 16-unit pipeline with PV one unit behind and
        # K/Q projections of later cts + out projection as PE filler.
        kq = [None] * NCT
        kq[0] = kq_proj(0)
        # V tiles 0-5 fill the PE while rope(ct0) runs on DVE; the rest of
        # V is spread over steps 0-1 so ACT's exp stream starts ~25us
        # earlier.  All of V must be emitted before PV(u0) (step 1 post).
        for m in range(0, 6):
            v_proj(m)

        units = [(ct, qh, half) for ct in range(NCT) for qh in range(2)
                 for half in range(2)]
        # filler emitted after scores_pv(u_i), keyed by pipeline step i.
        # All of V must precede step 1 (PV(u0) consumes every v tile).
        filler = {
            0: lambda: [v_proj(m) for m in range(6, KT_S)],
            1: lambda: kq.__setitem__(1, kq_proj(1)),
            4: lambda: kq.__setitem__(2, kq_proj(2)),
            8: lambda: kq.__setitem__(3, kq_proj(3)),
        }
        prev = None
        for i, (ct, qh, half) in enumerate(units):
            krA, krB, qr = kq[ct]
            u = (ct, qh, half, krA, krB, qr)
            expt = scores_pv(u, prev)
            if f := filler.get(i):
                f()
            prev = (u, expt)
        # O(m0-7) needs unit 13's normalize (emitted inside step 14) and
        # covers the exp latency of unit 15 before its PV runs.
        for m in range(0, 8):
            o_proj(m, evict_act=False)
        pv_last(*prev)
        for m in range(8, KT_S):
            o_proj(m, deep=True)

    nc.compile()
    return nc


_NC = None
LAST_RESULT = None
LAST_IN_MAPS = None


def _get_nc():
    global _NC
    if _NC is None:
        _NC = _build()
    return _NC


def unshard(results):
    """Sum the per-core bf16 partial outputs into the full [B, S, D]."""
    out_full = np.empty((B, S, D), np.float32)
    for b in range(B):
        out_full[b] = (results[2 * b]["out"].astype(np.float32) +
                       results[2 * b + 1]["out"].astype(np.float32))
    return out_full


def kernel(x, cos, sin, wq, wk, wv, wo):
    global LAST_RESULT, LAST_IN_MAPS
    x = np.asarray(x)
    cos = np.asarray(cos)
    sin = np.asarray(sin)

    def bf(a):
        return np.ascontiguousarray(a, dtype=np.float32).astype(BF_NP)

    cosT = cos[0, :, 0, :].T.astype(np.float32)   # [DH, S]
    sinT = sin[0, :, 0, :].T.astype(np.float32)
    sinr = np.concatenate([-sinT[:DH // 2], sinT[DH // 2:]], axis=0)
    cos2 = bf(np.concatenate([cosT, cosT], axis=0))   # [128, S]
    sinr2 = bf(np.concatenate([sinr, sinr], axis=0))

    wqT, wkT, wvT, woT = (w.T.astype(np.float32) for w in (wq, wk, wv, wo))
    in_maps = []
    for c in range(NCORES):
        b, hh = c // 2, c % 2
        d0 = hh * DL
        in_maps.append({
            "xT": bf(x[b].T),
            "wqT": bf(wqT[:, d0:d0 + DL]),
            "wkT": bf(wkT[:, d0:d0 + DL]),
            "wvT": bf(wvT[:, d0:d0 + DL]),
            "woT": bf(woT[d0:d0 + DL, :]),
            "cosk": cos2, "sinrk": sinr2,
        })

    LAST_IN_MAPS = in_maps
    nc = _get_nc()
    res = bass_utils.run_bass_kernel_spmd(nc, in_maps,
                                          core_ids=list(range(NCORES)))
    LAST_RESULT = res
    return unshard(res.results)
